# revision 27
# baseline (speedup 1.0000x reference)
"""MoE layer (8 experts, top-2) on 8 Trainium2 NeuronCores — D_FF-parallel.

Instead of one expert per core (which pads every core to the *largest*
expert's token count), every core owns a 512-wide slice of D_FF for ALL
8 experts and processes ALL routed token-pairs through its slice:

    h_c  = gelu(W1[e][:, c*512:(c+1)*512].T @ x + b1_slice)
    y_c  = W2[e][c*512:(c+1)*512, :].T @ h_c          (partial sum)
    y    = sum_c y_c                                   (host, float64)

Per-core weight bytes are identical to expert-parallel (16.8 MB) but the
token work is perfectly balanced: sum_e cap_e ~ 8200 slots instead of
8 * max_e cnt_e ~ 8544.  The program is uniform across cores (the tile
list depends only on global expert counts), so plain SPMD still works.
Partial outputs go back in bf16 (their sum adds ~0.3% relative error,
well under the 2e-2 gate); the very last tile's second half-pass is
copied out through two alternating engines so the post-matmul drain is
short.

Returns the full [B, S, D] float32 output.
"""

import os
import sys

for _p in ("/opt/trn_rl_repo",):
    if _p not in sys.path:
        sys.path.insert(0, _p)

import numpy as np
import ml_dtypes

import concourse.bass as bass
import concourse.mybir as mybir
import concourse.tile as tile
from concourse import bacc
from concourse.bass_utils import run_bass_kernel_spmd

D_MODEL = 1024
D_FF = 4096
NUM_EXPERTS = 8
TOP_K = 2
N_CORES = 8
P = 128          # SBUF partitions
DC = D_MODEL // P
F_SLICE = D_FF // N_CORES       # 512 ffn columns per core
F8C = F_SLICE // P              # 4 fc-chunks per core

N_WARMUP = 24

LAST_EXEC_NS = None


def _install_profile_hook():
    """Provide antenv.axon_hooks (NTFF profiling) if the image lacks it."""
    import types
    import contextlib
    import ctypes
    try:
        from antenv.axon_hooks import get_axon_ntff_profile_hook  # noqa: F401
        return
    except ImportError:
        pass
    so = "/opt/axon/libaxon_pjrt.so"
    if not os.path.exists(so):
        return
    lib = ctypes.CDLL(so)
    if not hasattr(lib, "axon_start_nrt_profile"):
        return
    lib.axon_start_nrt_profile.argtypes = [ctypes.POINTER(ctypes.c_int64),
                                           ctypes.c_size_t]
    lib.axon_start_nrt_profile.restype = ctypes.c_int64
    lib.axon_stop_nrt_profile.argtypes = [ctypes.c_char_p]
    lib.axon_stop_nrt_profile.restype = ctypes.c_int64

    @contextlib.contextmanager
    def _hook(output_dir, device_ids):
        import jax
        jax.devices()
        if device_ids:
            ids = (ctypes.c_int64 * len(device_ids))(*device_ids)
            rc = lib.axon_start_nrt_profile(ids, len(device_ids))
        else:
            rc = lib.axon_start_nrt_profile(None, 0)
        try:
            yield
        finally:
            if rc == 0:
                n = lib.axon_stop_nrt_profile(str(output_dir).encode())
                print(f"profile: {n} ntff file(s) -> {output_dir}",
                      file=sys.stderr)

    mod = types.ModuleType("antenv.axon_hooks")
    mod.get_axon_ntff_profile_hook = lambda: _hook
    mod.set_axon_ntff_profile_hook = lambda h: None
    sys.modules["antenv.axon_hooks"] = mod
    import antenv
    antenv.axon_hooks = mod
    import concourse.bass_utils as _bu
    _bu.upload_artifacts = lambda tmpdir: tmpdir


def _tile_shape(max_cnt):
    """Equal even tile size (<=512) and count covering max_cnt tokens."""
    lo = max(256, max_cnt)
    n = (lo + 511) // 512
    tn = -(-lo // n)
    tn += tn % 2
    return tn, n


def _plan(cnts):
    """Per-expert capacities and the flat tile list (same on every core).

    Experts are processed in descending tile-size order: the first tile's
    weight-demand rate is one fc-chunk (256 KB) per 8*tn cycles, and while
    the chip is still HAM-throttled the DMA rings only sustain
    ~150 GB/s — a large-tn expert first keeps demand below supply.  The
    smallest tile lands last, which also shortens the final drain.
    """
    shapes = [_tile_shape(c) for c in cnts]
    order = sorted(range(len(cnts)), key=lambda e: -shapes[e][0])
    caps = [None] * len(cnts)
    tiles = []      # (expert, slot_t0, tn)
    s = 0
    for e in order:
        tn, nt = shapes[e]
        caps[e] = (s, tn * nt, tn, nt)
        for i in range(nt):
            tiles.append((e, s + i * tn, tn))
        s += tn * nt
    return caps, tiles, s


def _build_program(caps, tiles, slots):
    """SPMD program: this core's F-slice of every expert over all tiles.

    DRAM layouts match SBUF exactly:
      xT  [P, DC, slots]      xT[p, dc, t]        = x[t, dc*128+p]
      W1  [E, P, F8C, DC, P]  W1[e,p,fc,dc,j] = W1[e][dc*128+p, o+fc*128+j]
      W2  [E, P, F8C, D]      W2[e,p,fc,d]    = W2[e][o+fc*128+p, d]
      b1  [P, E, F8C]         b1[p,e,fc]      = b1[e][o+fc*128+p]
    where o = core_id*512 is the F-slice offset (the host bakes it into
    each core's input map; the program is identical on every core).
    Output: yT [P, sum(DC*tn)] bf16 partials, one contiguous [P, DC*tn]
    block per tile (the host transposes blocks back to [D, tn]).
    """
    bf16 = mybir.dt.bfloat16
    f32 = mybir.dt.float32
    nc = bacc.Bacc("TRN2", target_bir_lowering=False, debug=False,
                   num_devices=N_CORES)

    tn_max = max(t[2] for t in tiles)
    first_e = tiles[0][0]
    e_order = []
    for e, _, _ in tiles:
        if e not in e_order:
            e_order.append(e)
    tn_e0 = caps[first_e][2]

    # x comes in as per-block contiguous tensors (a slice of one big
    # [P, DC, slots] tensor would DMA in 700-byte runs with 1000-row
    # descriptor tables — measured ~5x slower ring throughput).  Tile 0
    # arrives in four 2-dc-chunk pieces: the DMA rings are HAM-throttled
    # until ~3us after the warm-up starts, so the first matmul group is
    # gated by however few bytes it strictly needs.
    xT0_d = [nc.dram_tensor(f"xT0{q}", [P, 2, tn_e0], bf16,
                            kind="ExternalInput").ap()
             for q in range(DC // 2)]
    ne0r = caps[first_e][3] - 1      # remaining first-expert tiles
    xT0r_d = [nc.dram_tensor(f"xT0r{i}", [P, DC, tn_e0], bf16,
                             kind="ExternalInput").ap()
              for i in range(ne0r)]
    xe_d = {e: nc.dram_tensor(f"xe{e}", [P, DC, caps[e][1]], bf16,
                              kind="ExternalInput").ap()
            for e in e_order[1:]}
    w1_d = nc.dram_tensor("W1", [NUM_EXPERTS, P, F8C, DC, P], bf16,
                          kind="ExternalInput").ap()
    w2_d = nc.dram_tensor("W2", [NUM_EXPERTS, P, F8C, D_MODEL], bf16,
                          kind="ExternalInput").ap()
    b1_d = nc.dram_tensor("b1", [P, NUM_EXPERTS, F8C], f32,
                          kind="ExternalInput").ap()
    # outputs are written per-tile contiguous ([P, DC*tn] blocks packed
    # along the free dim) — a [D, slots] destination would mean 700-byte
    # runs and 1000-row descriptor tables per DMA, which crawls
    y_off = []
    o = 0
    for _, _, tn in tiles:
        y_off.append(o)
        o += DC * tn
    yT_d = nc.dram_tensor("yT", [P, o], bf16, kind="ExternalOutput").ap()

    with tile.TileContext(nc) as tc:
        with (
            tc.tile_pool(name="wpool", bufs=1) as wpool,
            tc.tile_pool(name="xpool", bufs=2) as xpool,
            tc.tile_pool(name="hpool", bufs=1) as hpool,
            tc.tile_pool(name="ypool", bufs=2) as ypool,
            tc.tile_pool(name="ph", bufs=2, space="PSUM") as ph_pool,
            tc.tile_pool(name="py", bufs=1, space="PSUM") as py_pool,
        ):
            # ACT ring: expert-0 x tile 0 in four 2-chunk pieces (they gate
            # the first matmul group), bias, rest of e0; y DMAs ride later
            xs0 = []
            for q in range(DC // 2):
                t = wpool.tile([P, 2, tn_e0], bf16, tag=f"xs0{q}",
                               name=f"xs0{q}")
                nc.scalar.dma_start(t[:], xT0_d[q])
                xs0.append(t)
            b1s = wpool.tile([P, NUM_EXPERTS, F8C], f32)
            nc.scalar.dma_start(b1s[:], b1_d)
            xe0r = []
            for i in range(ne0r):
                t = wpool.tile([P, DC, tn_e0], bf16, tag=f"xe0r{i}",
                               name=f"xe0r{i}")
                nc.scalar.dma_start(t[:], xT0r_d[i])
                xe0r.append(t)

            # SP ring: expert 0's weights graded (small first so compute
            # starts early), then per expert e>=1: x block, W1, W2 — each
            # bundle lands well before that expert's tile window
            w1q = {}
            w2q = {}
            fe = first_e
            w1q[fe] = wpool.tile([P, F8C, DC, P], bf16, tag="w1e0",
                                 name="w1e0")
            for fc in range(F8C):
                nc.sync.dma_start(w1q[fe][:, fc:fc + 1],
                                  w1_d[fe, :, fc:fc + 1])
            w2q[fe] = wpool.tile([P, F8C, D_MODEL], bf16, tag="w2e0",
                                 name="w2e0")
            nc.sync.dma_start(w2q[fe][:, :2], w2_d[fe, :, :2])
            nc.sync.dma_start(w2q[fe][:, 2:], w2_d[fe, :, 2:])
            xq = {}
            cap_max = max(caps[e][1] for e in e_order[1:])
            for e in e_order[1:]:
                s0, cap, _, _ = caps[e]
                xe = xpool.tile([P, DC, cap_max], bf16, tag="xe",
                                name=f"xe{e}")
                nc.sync.dma_start(xe[:, :, :cap], xe_d[e])
                xq[e] = xe
                w1q[e] = wpool.tile([P, F8C, DC, P], bf16, tag=f"w1e{e}",
                                    name=f"w1e{e}")
                nc.sync.dma_start(w1q[e][:], w1_d[e])
                w2q[e] = wpool.tile([P, F8C, D_MODEL], bf16, tag=f"w2e{e}",
                                    name=f"w2e{e}")
                nc.sync.dma_start(w2q[e][:], w2_d[e])

            def x_slice(e, t0, tn, dc):
                """x for slot range [t0, t0+tn) of expert e, chunk dc."""
                s0, cap, _, _ = caps[e]
                o = t0 - s0
                if e == first_e:
                    ti = o // tn_e0
                    if ti == 0:
                        return xs0[dc // 2][:, dc % 2, o:o + tn]
                    return xe0r[ti - 1][:, dc, :tn]
                return xq[e][:, dc, o:o + tn]

            # PE warm-up: dummy matmuls while the first loads land, so HAM
            # un-throttles right as the real stream begins (a gap between
            # warm-up and the first real matmul resets the ramp)
            warm = wpool.tile([P, 256], bf16)
            nc.vector.memset(warm[:], 0.0)
            wps, _ = tc.tile([P, 256], f32, space="PSUM", name="warmps")
            for _ in range(N_WARMUP):
                nc.tensor.matmul(wps[:], warm[:, :P], warm[:], start=True,
                                 stop=True)

            half = DC // 2
            for k, (e, t0, tn) in enumerate(tiles):
                yo = y_off[k]
                last = k == len(tiles) - 1
                # hT = gelu(W1_slice.T @ x + b1), layout [F(part), tokens]
                hT = hpool.tile([P, F8C, tn_max], bf16, tag="hT")
                for fc in range(F8C):
                    ph = ph_pool.tile([P, tn_max], f32, tag="ph")
                    for dc in range(DC):
                        nc.tensor.matmul(
                            ph[:, :tn],
                            w1q[e][:, fc, dc, :],
                            x_slice(e, t0, tn, dc),
                            start=(dc == 0),
                            stop=(dc == DC - 1),
                        )
                    nc.scalar.activation(
                        hT[:, fc, :tn], ph[:, :tn],
                        mybir.ActivationFunctionType.Gelu,
                        bias=b1s[:, e, fc:fc + 1], scale=1.0,
                    )

                # partial yT = W2_slice.T @ hT in two dc-halves, fc outer
                yt = ypool.tile([P, DC * tn_max], bf16, tag="yt")
                for h in range(2):
                    gp = 2 * k + h
                    dcs = range(h * half, (h + 1) * half)
                    pys = {dc: py_pool.tile([P, tn_max], f32,
                                            tag=f"py{(gp * 4 + i) % 5}",
                                            name=f"py_k{k}h{h}d{dc}")
                           for i, dc in enumerate(dcs)}
                    for fc in range(F8C):
                        for dc in dcs:
                            nc.tensor.matmul(
                                pys[dc][:, :tn],
                                w2q[e][:, fc, dc * P:(dc + 1) * P],
                                hT[:, fc, :tn],
                                start=(fc == 0),
                                stop=(fc == F8C - 1),
                            )
                    if last and h == 1:
                        # final half-pass: copies alternate between two
                        # engines and the DMA is split -> short drain
                        for i, dc in enumerate(dcs):
                            sl = slice(dc * tn, (dc + 1) * tn)
                            if i % 2 == 0:
                                nc.vector.tensor_copy(yt[:, sl],
                                                      pys[dc][:, :tn])
                            else:
                                nc.scalar.activation(
                                    yt[:, sl], pys[dc][:, :tn],
                                    mybir.ActivationFunctionType.Copy,
                                    scale=1.0)
                            if dc % 2 == 1:
                                nc.scalar.dma_start(
                                    yT_d[:, yo + (dc - 1) * tn:
                                         yo + (dc + 1) * tn],
                                    yt[:, (dc - 1) * tn:(dc + 1) * tn])
                    else:
                        for dc in dcs:
                            nc.vector.tensor_copy(
                                yt[:, dc * tn:(dc + 1) * tn],
                                pys[dc][:, :tn])
                        if h == 1:
                            nc.scalar.dma_start(yT_d[:, yo:yo + DC * tn],
                                                yt[:, :DC * tn])
                        elif last:
                            nc.scalar.dma_start(yT_d[:, yo:yo + half * tn],
                                                yt[:, :half * tn])

    nc.compile()
    return nc


def _route(x_flat, Wg):
    """Replicate the reference gate in float64: softmax, top-2, renorm."""
    logits = x_flat.astype(np.float64) @ Wg.astype(np.float64)
    logits -= logits.max(axis=-1, keepdims=True)
    p = np.exp(logits)
    p /= p.sum(axis=-1, keepdims=True)
    order = np.argsort(-p, axis=-1, kind="stable")[:, :TOP_K]   # [T, 2]
    rows = np.arange(p.shape[0])[:, None]
    tv = p[rows, order]                                          # [T, 2]
    tvn = tv / (tv.sum(axis=-1, keepdims=True) + 1e-8)
    return order, tvn


def kernel(x, Wg, W1, b1, W2, b2):
    global LAST_EXEC_NS
    x = np.asarray(x, dtype=np.float32)
    Wg = np.asarray(Wg, dtype=np.float32)
    W1 = np.asarray(W1, dtype=np.float32)
    b1 = np.asarray(b1, dtype=np.float32)
    W2 = np.asarray(W2, dtype=np.float32)
    b2 = np.asarray(b2, dtype=np.float32)

    B, S, D = x.shape
    x_flat = x.reshape(-1, D)
    T = x_flat.shape[0]

    order, tvn = _route(x_flat, Wg)

    idx = []
    wts = []
    for e in range(NUM_EXPERTS):
        sel = np.nonzero((order == e).any(axis=1))[0]
        idx.append(sel)
        wmat = np.where(order[sel] == e, tvn[sel], 0.0)
        wts.append(wmat.sum(axis=-1))                            # [cnt]

    caps, tiles, slots = _plan([len(s) for s in idx])
    tn_last = tiles[-1][2]

    # a Bass program object must not be re-run after lowering — build fresh
    # every call; the neuron compile cache keeps repeat builds fast
    nc = _build_program(caps, tiles, slots)

    bf16 = ml_dtypes.bfloat16
    xblocks = {}
    first_e = tiles[0][0]
    tn_e0 = caps[first_e][2]
    for e in range(NUM_EXPERTS):
        cap = caps[e][1]
        sel = idx[e]
        xe = np.zeros((P, DC, cap), dtype=bf16)
        xe[:, :, :len(sel)] = \
            x_flat[sel].reshape(-1, DC, P).transpose(2, 1, 0)
        if e == first_e:
            for q in range(DC // 2):
                xblocks[f"xT0{q}"] = np.ascontiguousarray(
                    xe[:, 2 * q:2 * q + 2, :tn_e0])
            for i in range(caps[e][3] - 1):
                xblocks[f"xT0r{i}"] = np.ascontiguousarray(
                    xe[:, :, (i + 1) * tn_e0:(i + 2) * tn_e0])
        else:
            xblocks[f"xe{e}"] = np.ascontiguousarray(xe)

    in_maps = []
    for c in range(N_CORES):
        o = c * F_SLICE
        # [E, D, 512] -> [E, DC, P, F8C, 128] -> [E, P, F8C, DC, 128]
        w1c = np.ascontiguousarray(
            W1[:, :, o:o + F_SLICE]
            .reshape(NUM_EXPERTS, DC, P, F8C, P)
            .transpose(0, 2, 3, 1, 4)).astype(bf16)
        # [E, 512, D] -> [E, F8C, P, D] -> [E, P, F8C, D]
        w2c = np.ascontiguousarray(
            W2[:, o:o + F_SLICE, :]
            .reshape(NUM_EXPERTS, F8C, P, D_MODEL)
            .transpose(0, 2, 1, 3)).astype(bf16)
        # [E, 512] -> [E, F8C, P] -> [P, E, F8C]
        b1c = np.ascontiguousarray(
            b1[:, o:o + F_SLICE].reshape(NUM_EXPERTS, F8C, P)
            .transpose(2, 0, 1))
        in_maps.append({"W1": w1c, "W2": w2c, "b1": b1c, **xblocks})

    trace = bool(os.environ.get("MOE_TRACE"))
    _install_profile_hook()   # also covers a harness-set BASS_TRACE=1
    try:
        res = run_bass_kernel_spmd(
            nc, in_maps, list(range(N_CORES)),
            trace=trace,
            tmpdir=os.environ.get("MOE_TRACE_DIR") or None,
        )
    except Exception:
        if not (trace or os.environ.get("BASS_TRACE")):
            raise
        os.environ["BASS_NEVER_TRACE"] = "1"
        res = run_bass_kernel_spmd(nc, in_maps, list(range(N_CORES)))
    LAST_EXEC_NS = res.exec_time_ns

    # sum the 8 partial outputs (float64), unpacking the per-tile blocks
    ysum = np.zeros((D_MODEL, slots), dtype=np.float64)
    for c in range(N_CORES):
        yp = np.asarray(res.results[c]["yT"])     # [P, sum(DC*tn)] bf16
        o = 0
        for k, (e, t0, tn) in enumerate(tiles):
            # block [P, DC, tn] -> rows d = dc*128+p
            blk = yp[:, o:o + DC * tn].astype(np.float64)
            o += DC * tn
            blk = blk.reshape(P, DC, tn).transpose(1, 0, 2).reshape(
                D_MODEL, tn)
            ysum[:, t0:t0 + tn] += blk

    out = np.zeros((T, D_MODEL), dtype=np.float64)
    for e in range(NUM_EXPERTS):
        s0 = caps[e][0]
        sel = idx[e]
        y = ysum[:, s0:s0 + len(sel)].T
        out[sel] += wts[e][:, None] * (y + b2[e].astype(np.float64))

    return out.reshape(B, S, D_MODEL).astype(np.float32)


# revision 28
# speedup vs baseline: 1.0063x; 1.0063x over previous
"""MoE layer (8 experts, top-2) on 8 Trainium2 NeuronCores — D_FF-parallel.

Instead of one expert per core (which pads every core to the *largest*
expert's token count), every core owns a 512-wide slice of D_FF for ALL
8 experts and processes ALL routed token-pairs through its slice:

    h_c  = gelu(W1[e][:, c*512:(c+1)*512].T @ x + b1_slice)
    y_c  = W2[e][c*512:(c+1)*512, :].T @ h_c          (partial sum)
    y    = sum_c y_c                                   (host, float64)

Per-core weight bytes are identical to expert-parallel (16.8 MB) but the
token work is perfectly balanced: sum_e cap_e ~ 8200 slots instead of
8 * max_e cnt_e ~ 8544.  The program is uniform across cores (the tile
list depends only on global expert counts), so plain SPMD still works.
Partial outputs go back in bf16 (their sum adds ~0.3% relative error,
well under the 2e-2 gate); the very last tile's second half-pass is
copied out through two alternating engines so the post-matmul drain is
short.

Returns the full [B, S, D] float32 output.
"""

import os
import sys

for _p in ("/opt/trn_rl_repo",):
    if _p not in sys.path:
        sys.path.insert(0, _p)

import numpy as np
import ml_dtypes

import concourse.bass as bass
import concourse.mybir as mybir
import concourse.tile as tile
from concourse import bacc
from concourse.bass_utils import run_bass_kernel_spmd

D_MODEL = 1024
D_FF = 4096
NUM_EXPERTS = 8
TOP_K = 2
N_CORES = 8
P = 128          # SBUF partitions
DC = D_MODEL // P
F_SLICE = D_FF // N_CORES       # 512 ffn columns per core
F8C = F_SLICE // P              # 4 fc-chunks per core

N_WARMUP = 30

LAST_EXEC_NS = None


def _install_profile_hook():
    """Provide antenv.axon_hooks (NTFF profiling) if the image lacks it."""
    import types
    import contextlib
    import ctypes
    try:
        from antenv.axon_hooks import get_axon_ntff_profile_hook  # noqa: F401
        return
    except ImportError:
        pass
    so = "/opt/axon/libaxon_pjrt.so"
    if not os.path.exists(so):
        return
    lib = ctypes.CDLL(so)
    if not hasattr(lib, "axon_start_nrt_profile"):
        return
    lib.axon_start_nrt_profile.argtypes = [ctypes.POINTER(ctypes.c_int64),
                                           ctypes.c_size_t]
    lib.axon_start_nrt_profile.restype = ctypes.c_int64
    lib.axon_stop_nrt_profile.argtypes = [ctypes.c_char_p]
    lib.axon_stop_nrt_profile.restype = ctypes.c_int64

    @contextlib.contextmanager
    def _hook(output_dir, device_ids):
        import jax
        jax.devices()
        if device_ids:
            ids = (ctypes.c_int64 * len(device_ids))(*device_ids)
            rc = lib.axon_start_nrt_profile(ids, len(device_ids))
        else:
            rc = lib.axon_start_nrt_profile(None, 0)
        try:
            yield
        finally:
            if rc == 0:
                n = lib.axon_stop_nrt_profile(str(output_dir).encode())
                print(f"profile: {n} ntff file(s) -> {output_dir}",
                      file=sys.stderr)

    mod = types.ModuleType("antenv.axon_hooks")
    mod.get_axon_ntff_profile_hook = lambda: _hook
    mod.set_axon_ntff_profile_hook = lambda h: None
    sys.modules["antenv.axon_hooks"] = mod
    import antenv
    antenv.axon_hooks = mod
    import concourse.bass_utils as _bu
    _bu.upload_artifacts = lambda tmpdir: tmpdir


def _tile_shape(max_cnt):
    """Equal even tile size (<=512) and count covering max_cnt tokens."""
    lo = max(256, max_cnt)
    n = (lo + 511) // 512
    tn = -(-lo // n)
    tn += tn % 2
    return tn, n


def _plan(cnts):
    """Per-expert capacities and the flat tile list (same on every core).

    Experts are processed in descending tile-size order: the first tile's
    weight-demand rate is one fc-chunk (256 KB) per 8*tn cycles, and while
    the chip is still HAM-throttled the DMA rings only sustain
    ~150 GB/s — a large-tn expert first keeps demand below supply.  The
    smallest tile lands last, which also shortens the final drain.
    """
    shapes = [_tile_shape(c) for c in cnts]
    order = sorted(range(len(cnts)), key=lambda e: -shapes[e][0])
    caps = [None] * len(cnts)
    tiles = []      # (expert, slot_t0, tn)
    s = 0
    for e in order:
        tn, nt = shapes[e]
        caps[e] = (s, tn * nt, tn, nt)
        for i in range(nt):
            tiles.append((e, s + i * tn, tn))
        s += tn * nt
    return caps, tiles, s


def _build_program(caps, tiles, slots):
    """SPMD program: this core's F-slice of every expert over all tiles.

    DRAM layouts match SBUF exactly:
      xT  [P, DC, slots]      xT[p, dc, t]        = x[t, dc*128+p]
      W1  [E, P, F8C, DC, P]  W1[e,p,fc,dc,j] = W1[e][dc*128+p, o+fc*128+j]
      W2  [E, P, F8C, D]      W2[e,p,fc,d]    = W2[e][o+fc*128+p, d]
      b1  [P, E, F8C]         b1[p,e,fc]      = b1[e][o+fc*128+p]
    where o = core_id*512 is the F-slice offset (the host bakes it into
    each core's input map; the program is identical on every core).
    Output: yT [P, sum(DC*tn)] bf16 partials, one contiguous [P, DC*tn]
    block per tile (the host transposes blocks back to [D, tn]).
    """
    bf16 = mybir.dt.bfloat16
    f32 = mybir.dt.float32
    nc = bacc.Bacc("TRN2", target_bir_lowering=False, debug=False,
                   num_devices=N_CORES)

    tn_max = max(t[2] for t in tiles)
    first_e = tiles[0][0]
    e_order = []
    for e, _, _ in tiles:
        if e not in e_order:
            e_order.append(e)
    tn_e0 = caps[first_e][2]

    # x comes in as per-block contiguous tensors (a slice of one big
    # [P, DC, slots] tensor would DMA in 700-byte runs with 1000-row
    # descriptor tables — measured ~5x slower ring throughput).  Tile 0
    # arrives in four 2-dc-chunk pieces: the DMA rings are HAM-throttled
    # until ~3us after the warm-up starts, so the first matmul group is
    # gated by however few bytes it strictly needs.
    xT0_d = [nc.dram_tensor(f"xT0{q}", [P, 2, tn_e0], bf16,
                            kind="ExternalInput").ap()
             for q in range(DC // 2)]
    ne0r = caps[first_e][3] - 1      # remaining first-expert tiles
    xT0r_d = [nc.dram_tensor(f"xT0r{i}", [P, DC, tn_e0], bf16,
                             kind="ExternalInput").ap()
              for i in range(ne0r)]
    xe_d = {e: nc.dram_tensor(f"xe{e}", [P, DC, caps[e][1]], bf16,
                              kind="ExternalInput").ap()
            for e in e_order[1:]}
    w1_d = nc.dram_tensor("W1", [NUM_EXPERTS, P, F8C, DC, P], bf16,
                          kind="ExternalInput").ap()
    w2_d = nc.dram_tensor("W2", [NUM_EXPERTS, P, F8C, D_MODEL], bf16,
                          kind="ExternalInput").ap()
    b1_d = nc.dram_tensor("b1", [P, NUM_EXPERTS, F8C], f32,
                          kind="ExternalInput").ap()
    # outputs are written per-tile contiguous ([P, DC*tn] blocks packed
    # along the free dim) — a [D, slots] destination would mean 700-byte
    # runs and 1000-row descriptor tables per DMA, which crawls
    y_off = []
    o = 0
    for _, _, tn in tiles:
        y_off.append(o)
        o += DC * tn
    yT_d = nc.dram_tensor("yT", [P, o], bf16, kind="ExternalOutput").ap()

    with tile.TileContext(nc) as tc:
        with (
            tc.tile_pool(name="wpool", bufs=1) as wpool,
            tc.tile_pool(name="xpool", bufs=2) as xpool,
            tc.tile_pool(name="hpool", bufs=1) as hpool,
            tc.tile_pool(name="ypool", bufs=2) as ypool,
            tc.tile_pool(name="ph", bufs=2, space="PSUM") as ph_pool,
            tc.tile_pool(name="py", bufs=1, space="PSUM") as py_pool,
        ):
            # ACT ring: expert-0 x tile 0 in four 2-chunk pieces (they gate
            # the first matmul group), bias, rest of e0; y DMAs ride later
            xs0 = []
            for q in range(DC // 2):
                t = wpool.tile([P, 2, tn_e0], bf16, tag=f"xs0{q}",
                               name=f"xs0{q}")
                nc.scalar.dma_start(t[:], xT0_d[q])
                xs0.append(t)
            b1s = wpool.tile([P, NUM_EXPERTS, F8C], f32)
            nc.scalar.dma_start(b1s[:], b1_d)
            xe0r = []
            for i in range(ne0r):
                t = wpool.tile([P, DC, tn_e0], bf16, tag=f"xe0r{i}",
                               name=f"xe0r{i}")
                nc.scalar.dma_start(t[:], xT0r_d[i])
                xe0r.append(t)

            # SP ring: expert 0's weights graded (small first so compute
            # starts early), then per expert e>=1: x block, W1, W2 — each
            # bundle lands well before that expert's tile window
            w1q = {}
            w2q = {}
            fe = first_e
            w1q[fe] = wpool.tile([P, F8C, DC, P], bf16, tag="w1e0",
                                 name="w1e0")
            for fc in range(F8C):
                nc.sync.dma_start(w1q[fe][:, fc:fc + 1],
                                  w1_d[fe, :, fc:fc + 1])
            w2q[fe] = wpool.tile([P, F8C, D_MODEL], bf16, tag="w2e0",
                                 name="w2e0")
            nc.sync.dma_start(w2q[fe][:, :2], w2_d[fe, :, :2])
            nc.sync.dma_start(w2q[fe][:, 2:], w2_d[fe, :, 2:])
            xq = {}
            cap_max = max(caps[e][1] for e in e_order[1:])
            for e in e_order[1:]:
                s0, cap, _, _ = caps[e]
                xe = xpool.tile([P, DC, cap_max], bf16, tag="xe",
                                name=f"xe{e}")
                nc.sync.dma_start(xe[:, :, :cap], xe_d[e])
                xq[e] = xe
                w1q[e] = wpool.tile([P, F8C, DC, P], bf16, tag=f"w1e{e}",
                                    name=f"w1e{e}")
                nc.sync.dma_start(w1q[e][:], w1_d[e])
                w2q[e] = wpool.tile([P, F8C, D_MODEL], bf16, tag=f"w2e{e}",
                                    name=f"w2e{e}")
                nc.sync.dma_start(w2q[e][:], w2_d[e])

            def x_slice(e, t0, tn, dc):
                """x for slot range [t0, t0+tn) of expert e, chunk dc."""
                s0, cap, _, _ = caps[e]
                o = t0 - s0
                if e == first_e:
                    ti = o // tn_e0
                    if ti == 0:
                        return xs0[dc // 2][:, dc % 2, o:o + tn]
                    return xe0r[ti - 1][:, dc, :tn]
                return xq[e][:, dc, o:o + tn]

            # PE warm-up: dummy matmuls while the first loads land, so HAM
            # un-throttles right as the real stream begins (a gap between
            # warm-up and the first real matmul resets the ramp)
            warm = wpool.tile([P, 256], bf16)
            nc.vector.memset(warm[:], 0.0)
            wps, _ = tc.tile([P, 256], f32, space="PSUM", name="warmps")
            for _ in range(N_WARMUP):
                nc.tensor.matmul(wps[:], warm[:, :P], warm[:], start=True,
                                 stop=True)

            half = DC // 2
            for k, (e, t0, tn) in enumerate(tiles):
                yo = y_off[k]
                last = k == len(tiles) - 1
                # hT = gelu(W1_slice.T @ x + b1), layout [F(part), tokens]
                hT = hpool.tile([P, F8C, tn_max], bf16, tag="hT")
                for fc in range(F8C):
                    ph = ph_pool.tile([P, tn_max], f32, tag="ph")
                    for dc in range(DC):
                        nc.tensor.matmul(
                            ph[:, :tn],
                            w1q[e][:, fc, dc, :],
                            x_slice(e, t0, tn, dc),
                            start=(dc == 0),
                            stop=(dc == DC - 1),
                        )
                    nc.scalar.activation(
                        hT[:, fc, :tn], ph[:, :tn],
                        mybir.ActivationFunctionType.Gelu,
                        bias=b1s[:, e, fc:fc + 1], scale=1.0,
                    )

                # partial yT = W2_slice.T @ hT in two dc-halves, fc outer
                yt = ypool.tile([P, DC * tn_max], bf16, tag="yt")
                for h in range(2):
                    gp = 2 * k + h
                    dcs = range(h * half, (h + 1) * half)
                    pys = {dc: py_pool.tile([P, tn_max], f32,
                                            tag=f"py{(gp * 4 + i) % 5}",
                                            name=f"py_k{k}h{h}d{dc}")
                           for i, dc in enumerate(dcs)}
                    for fc in range(F8C):
                        for dc in dcs:
                            nc.tensor.matmul(
                                pys[dc][:, :tn],
                                w2q[e][:, fc, dc * P:(dc + 1) * P],
                                hT[:, fc, :tn],
                                start=(fc == 0),
                                stop=(fc == F8C - 1),
                            )
                    if last and h == 1:
                        # final half-pass: copies alternate between two
                        # engines and the DMA is split -> short drain
                        for i, dc in enumerate(dcs):
                            sl = slice(dc * tn, (dc + 1) * tn)
                            if i % 2 == 0:
                                nc.vector.tensor_copy(yt[:, sl],
                                                      pys[dc][:, :tn])
                            else:
                                nc.scalar.activation(
                                    yt[:, sl], pys[dc][:, :tn],
                                    mybir.ActivationFunctionType.Copy,
                                    scale=1.0)
                            if dc % 2 == 1:
                                nc.scalar.dma_start(
                                    yT_d[:, yo + (dc - 1) * tn:
                                         yo + (dc + 1) * tn],
                                    yt[:, (dc - 1) * tn:(dc + 1) * tn])
                    else:
                        for dc in dcs:
                            nc.vector.tensor_copy(
                                yt[:, dc * tn:(dc + 1) * tn],
                                pys[dc][:, :tn])
                        if h == 1:
                            nc.scalar.dma_start(yT_d[:, yo:yo + DC * tn],
                                                yt[:, :DC * tn])
                        elif last:
                            nc.scalar.dma_start(yT_d[:, yo:yo + half * tn],
                                                yt[:, :half * tn])

    nc.compile()
    return nc


def _route(x_flat, Wg):
    """Replicate the reference gate in float64: softmax, top-2, renorm."""
    logits = x_flat.astype(np.float64) @ Wg.astype(np.float64)
    logits -= logits.max(axis=-1, keepdims=True)
    p = np.exp(logits)
    p /= p.sum(axis=-1, keepdims=True)
    order = np.argsort(-p, axis=-1, kind="stable")[:, :TOP_K]   # [T, 2]
    rows = np.arange(p.shape[0])[:, None]
    tv = p[rows, order]                                          # [T, 2]
    tvn = tv / (tv.sum(axis=-1, keepdims=True) + 1e-8)
    return order, tvn


def kernel(x, Wg, W1, b1, W2, b2):
    global LAST_EXEC_NS
    x = np.asarray(x, dtype=np.float32)
    Wg = np.asarray(Wg, dtype=np.float32)
    W1 = np.asarray(W1, dtype=np.float32)
    b1 = np.asarray(b1, dtype=np.float32)
    W2 = np.asarray(W2, dtype=np.float32)
    b2 = np.asarray(b2, dtype=np.float32)

    B, S, D = x.shape
    x_flat = x.reshape(-1, D)
    T = x_flat.shape[0]

    order, tvn = _route(x_flat, Wg)

    idx = []
    wts = []
    for e in range(NUM_EXPERTS):
        sel = np.nonzero((order == e).any(axis=1))[0]
        idx.append(sel)
        wmat = np.where(order[sel] == e, tvn[sel], 0.0)
        wts.append(wmat.sum(axis=-1))                            # [cnt]

    caps, tiles, slots = _plan([len(s) for s in idx])
    tn_last = tiles[-1][2]

    # a Bass program object must not be re-run after lowering — build fresh
    # every call; the neuron compile cache keeps repeat builds fast
    nc = _build_program(caps, tiles, slots)

    bf16 = ml_dtypes.bfloat16
    xblocks = {}
    first_e = tiles[0][0]
    tn_e0 = caps[first_e][2]
    for e in range(NUM_EXPERTS):
        cap = caps[e][1]
        sel = idx[e]
        xe = np.zeros((P, DC, cap), dtype=bf16)
        xe[:, :, :len(sel)] = \
            x_flat[sel].reshape(-1, DC, P).transpose(2, 1, 0)
        if e == first_e:
            for q in range(DC // 2):
                xblocks[f"xT0{q}"] = np.ascontiguousarray(
                    xe[:, 2 * q:2 * q + 2, :tn_e0])
            for i in range(caps[e][3] - 1):
                xblocks[f"xT0r{i}"] = np.ascontiguousarray(
                    xe[:, :, (i + 1) * tn_e0:(i + 2) * tn_e0])
        else:
            xblocks[f"xe{e}"] = np.ascontiguousarray(xe)

    in_maps = []
    for c in range(N_CORES):
        o = c * F_SLICE
        # [E, D, 512] -> [E, DC, P, F8C, 128] -> [E, P, F8C, DC, 128]
        w1c = np.ascontiguousarray(
            W1[:, :, o:o + F_SLICE]
            .reshape(NUM_EXPERTS, DC, P, F8C, P)
            .transpose(0, 2, 3, 1, 4)).astype(bf16)
        # [E, 512, D] -> [E, F8C, P, D] -> [E, P, F8C, D]
        w2c = np.ascontiguousarray(
            W2[:, o:o + F_SLICE, :]
            .reshape(NUM_EXPERTS, F8C, P, D_MODEL)
            .transpose(0, 2, 1, 3)).astype(bf16)
        # [E, 512] -> [E, F8C, P] -> [P, E, F8C]
        b1c = np.ascontiguousarray(
            b1[:, o:o + F_SLICE].reshape(NUM_EXPERTS, F8C, P)
            .transpose(2, 0, 1))
        in_maps.append({"W1": w1c, "W2": w2c, "b1": b1c, **xblocks})

    trace = bool(os.environ.get("MOE_TRACE"))
    _install_profile_hook()   # also covers a harness-set BASS_TRACE=1
    try:
        res = run_bass_kernel_spmd(
            nc, in_maps, list(range(N_CORES)),
            trace=trace,
            tmpdir=os.environ.get("MOE_TRACE_DIR") or None,
        )
    except Exception:
        if not (trace or os.environ.get("BASS_TRACE")):
            raise
        os.environ["BASS_NEVER_TRACE"] = "1"
        res = run_bass_kernel_spmd(nc, in_maps, list(range(N_CORES)))
    LAST_EXEC_NS = res.exec_time_ns

    # sum the 8 partial outputs (float64), unpacking the per-tile blocks
    ysum = np.zeros((D_MODEL, slots), dtype=np.float64)
    for c in range(N_CORES):
        yp = np.asarray(res.results[c]["yT"])     # [P, sum(DC*tn)] bf16
        o = 0
        for k, (e, t0, tn) in enumerate(tiles):
            # block [P, DC, tn] -> rows d = dc*128+p
            blk = yp[:, o:o + DC * tn].astype(np.float64)
            o += DC * tn
            blk = blk.reshape(P, DC, tn).transpose(1, 0, 2).reshape(
                D_MODEL, tn)
            ysum[:, t0:t0 + tn] += blk

    out = np.zeros((T, D_MODEL), dtype=np.float64)
    for e in range(NUM_EXPERTS):
        s0 = caps[e][0]
        sel = idx[e]
        y = ysum[:, s0:s0 + len(sel)].T
        out[sel] += wts[e][:, None] * (y + b2[e].astype(np.float64))

    return out.reshape(B, S, D_MODEL).astype(np.float32)
